# revision 48
# baseline (speedup 1.0000x reference)
"""Trainium2 Bass kernel for CausalSelfAttention (B=4, T=2048, C=1024, H=16)
with additive prev-prob key bias.

Sharding: 8 cores = data-parallel over B (4) x tensor-parallel over head
halves (2).  Each core computes qkv for its 8 heads, causal attention, and a
partial output projection (row-parallel W_proj); host sums the partials
per batch at unshard time.  Each core emits three partial tensors: `out`
(all rows; final-chunk rows carry only pairs 0-1), `out2` (final-chunk
rows, pair 2) and `out3` (final-chunk rows, pair 3).

Per-core device algorithm (v3 — phase-balanced schedule):
  - All matmul operands are bf16 (PSUM accumulation stays fp32).
  - K^T and Q^T kept feature-major with head pairs stacked in the 128
    partitions, so QK^T runs as two K=64 matmuls co-issued on row-group
    halves — full-array throughput despite K=64.
  - Scores are computed transposed (keys on partitions): softmax denominator
    comes from an extra EA column appended to V (M=65 PV matmuls), where
    EA[k] = (prev_probs[k]+1e-10)**-EPS folds the additive log bias into a
    multiplicative per-key scale of exp(qk/8).
  - Causality: block-trimmed matmul widths + one 128x128 triangular mask
    multiply (bf16, DVE 2x rate) per diagonal block.
  - PHASE BALANCE: the scalar/ACT engine (exp) is the pole in late query
    chunks (qc=3 needs ~64us of exp vs ~37us of attention matmul), while
    early chunks are PE-bound.  PE work is deferred into late chunks:
    K/V generation for chunks 2-3 is pulled into those chunks' own
    attention (deadline-forced just before ki crosses into the new key
    tiles), proj(1)/proj(2) run during qc=3, and the final chunk's
    projection is split per pair group: pairs 0-1 mid-qc3 into `out`,
    pair 2 into `out2` while pair 3's attention runs, pair 3 in the tail
    into `out3` as K=64 matmuls (so the B-half never needs the partition-
    shift DMA).  Filler generators yield every ~2 matmuls so the exp
    stream is never starved behind a coarse filler burst, and dry pulls
    emit dependency-free warm matmuls to keep the HAM clock at 2.4GHz.
  - Startup: weights are DMAed in Q|K|V column phases and chunk-0 QKV
    runs c-major across 4 concurrent PSUM groups so the PE tracks DMA
    arrival densely.
  - 1/denominator for most pairs: bounce the den row through DRAM on the
    gpsimd SWDGE queue; the dependent stack multiplies also run on gpsimd
    so the ~10us latency never blocks the DVE.  The very last pair
    computes 1/den on-chip (K=1 spread matmuls + PE transpose + rank-1
    broadcast) so the tail isn't gated by DMA round trips.
"""

import math
from contextlib import ExitStack

import ml_dtypes
import numpy as np

import concourse.bass as bass
import concourse.tile as tile
from concourse import bacc, mybir

F32 = mybir.dt.float32
MMD = mybir.dt.bfloat16
NPMMD = ml_dtypes.bfloat16

B, T, C, H = 4, 2048, 1024, 16
HD = C // H          # 64
NCORES = 8
HPC = H // 2         # 8 heads per core
FPC = HPC * HD       # 512 features per core
NKT = T // 128       # 16 key tiles
NQC = T // 512       # 4 query chunks (also the x t-chunks)
NCT = C // 128       # 8 contraction tiles
EPS_BIAS = 0.1
SCALE = 1.0 / math.sqrt(HD)


def build(tc, out_ap, out2_ap, out3_ap, xT, wqkv, wproj, ea,
          tri_dram, dsc1, dsc2):
    """Emit the per-core kernel into TileContext tc."""
    nc = tc.nc
    ctx = tc.ctx
    Exp = mybir.ActivationFunctionType.Exp
    Copy = mybir.ActivationFunctionType.Copy

    const = ctx.enter_context(tc.tile_pool(name="const", bufs=1))
    xs_pool = ctx.enter_context(tc.tile_pool(name="xs", bufs=16))
    qt_pool = ctx.enter_context(tc.tile_pool(name="qt", bufs=5))
    se_pool = ctx.enter_context(tc.tile_pool(name="se", bufs=4))
    tmp_pool = ctx.enter_context(tc.tile_pool(name="tmp", bufs=6))
    rec_pool = ctx.enter_context(tc.tile_pool(name="rec", bufs=4))
    scale_pool = ctx.enter_context(tc.tile_pool(name="scale", bufs=4))
    stack_pool = ctx.enter_context(tc.tile_pool(name="stack", bufs=12))
    pout_pool = ctx.enter_context(tc.tile_pool(name="pout", bufs=6))

    ps_pool = ctx.enter_context(tc.tile_pool(name="ps", bufs=2, space="PSUM"))
    st_pool = ctx.enter_context(tc.tile_pool(name="st", bufs=2, space="PSUM"))
    y_pool = ctx.enter_context(tc.tile_pool(name="y", bufs=2, space="PSUM"))

    # ---- constants / persistent buffers ----
    tri = const.tile([128, 128], MMD, name="tri")
    eacol = const.tile([128, NKT], F32, name="eacol")
    wp_sb = const.tile([128, FPC // 128, C], MMD, name="wp_sb")     # 8KB/p
    wp3b = const.tile([64, C], MMD, name="wp3b")  # W_proj rows 448:512 at base 0

    # small consts ride the scalar hw queue so the sync queue starts on
    # chunk-0 x tiles immediately
    nc.scalar.dma_start(out=eacol, in_=ea.rearrange("(k p) -> p k", p=128))
    nc.scalar.dma_start(out=tri, in_=tri_dram[:, :])

    oneb = const.tile([128, 1], MMD, name="oneb")
    nc.vector.memset(oneb, 1.0)

    wq_sb = const.tile([128, NCT, 3 * FPC], MMD, name="wq_sb")      # 24KB/p
    wqkv3 = wqkv.rearrange("(c p) f -> p c f", p=128)
    for part in range(3):  # Q columns, then K, then V
        for c in range(NCT):
            nc.gpsimd.dma_start(
                out=wq_sb[:, c, part * FPC:(part + 1) * FPC],
                in_=wqkv3[:, c, part * FPC:(part + 1) * FPC],
            )

    kt = const.tile([128, HPC // 2, T], MMD, name="kt")             # 16KB/p
    v2 = const.tile([128, NKT, HPC, HD + 1], MMD, name="v2")        # 16.6KB/p
    ones8 = const.tile([128, HPC], F32, name="ones8")
    nc.vector.memset(ones8, 1.0)
    warm = const.tile([128, 512], MMD, name="warm")
    nc.vector.memset(warm, 0.5)

    for kt_i in range(NKT):
        nc.vector.tensor_scalar(
            out=v2[:, kt_i, :, HD:HD + 1],
            in0=ones8.unsqueeze(2),
            scalar1=eacol[:, kt_i:kt_i + 1],
            scalar2=None,
            op0=mybir.AluOpType.mult,
        )

    qts_store = {}
    xs_store = {}

    def emit_warm(n=1):
        wps = ps_pool.tile([128, 512], F32, tag="ps", name="warmps")
        for _ in range(n):
            nc.tensor.matmul(wps, warm[:, 0:128], warm,
                             start=True, stop=True, skip_group_check=True)

    def emit_xs(qc):
        xs_tiles = []
        for c in range(NCT):
            xs = xs_pool.tile([128, 512], MMD, tag="xs", name=f"xs_{qc}_{c}")
            nc.sync.dma_start(
                out=xs, in_=xT[c * 128:(c + 1) * 128, qc * 512:(qc + 1) * 512]
            )
            xs_tiles.append(xs)
        xs_store[qc] = xs_tiles

    # ---- chunk 0 QKV: fully c-major across 4 concurrent PSUM groups so
    # the PE tracks the weight-phase DMA arrival densely. ----
    emit_xs(0)

    # dep-free warm burst: trips the HAM activity monitor to full clock
    # while the first weight/x tiles land
    emit_warm(8)

    xs0 = xs_store[0]

    def four_slots(pref):
        return [
            ps_pool.tile([128, 512], F32, tag="ps", name=f"{pref}_0"),
            ps_pool.tile([128, 512], F32, tag="ps", name=f"{pref}_1"),
            st_pool.tile([128, 512], F32, tag="st", name=f"{pref}_2"),
            y_pool.tile([128, 512], F32, tag="y", name=f"{pref}_3"),
        ]

    qps = four_slots("q0")
    for c in range(NCT):
        for p in range(HPC // 2):
            nc.tensor.matmul(
                qps[p], wq_sb[:, c, p * 128:(p + 1) * 128], xs0[c],
                start=(c == 0), stop=(c == NCT - 1),
            )
    qts0 = []
    for p in range(HPC // 2):
        qt = qt_pool.tile([128, 512], MMD, tag="qt", name=f"qt_0_{p}")
        nc.vector.tensor_copy(qt, qps[p])
        qts0.append(qt)
    qts_store[0] = qts0

    kps = four_slots("k0")
    for c in range(NCT):
        for p in range(HPC // 2):
            nc.tensor.matmul(
                kps[p], wq_sb[:, c, FPC + p * 128:FPC + (p + 1) * 128], xs0[c],
                start=(c == 0), stop=(c == NCT - 1),
            )
    for p in range(HPC // 2):
        nc.vector.tensor_copy(kt[:, p, 0:512], kps[p])

    # V0 only — V1..V3 are deferred into qc0's attention as deadline-
    # forced fillers so the first exps start ~5us earlier
    vps0 = ps_pool.tile([128, 512], F32, tag="ps", name="v0_0")
    for c in range(NCT):
        nc.tensor.matmul(
            vps0, xs0[c][:, 0:128], wq_sb[:, c, 2 * FPC:3 * FPC],
            start=(c == 0), stop=(c == NCT - 1),
        )
    nc.vector.tensor_scalar(
        out=v2[:, 0, :, 0:HD],
        in0=vps0.rearrange("p (h d) -> p h d", h=HPC),
        scalar1=eacol[:, 0:1],
        scalar2=None,
        op0=mybir.AluOpType.mult,
    )

    def gen_v0rest():
        for j in range(1, 4):
            ps = ps_pool.tile([128, 512], F32, tag="ps", name=f"vps_0_{j}")
            for c in range(NCT):
                nc.tensor.matmul(
                    ps, xs0[c][:, j * 128:(j + 1) * 128],
                    wq_sb[:, c, 2 * FPC:3 * FPC],
                    start=(c == 0), stop=(c == NCT - 1),
                )
                if c % 2 == 1 and c < NCT - 1:
                    yield
            nc.vector.tensor_scalar(
                out=v2[:, j, :, 0:HD],
                in0=ps.rearrange("p (h d) -> p h d", h=HPC),
                scalar1=eacol[:, j:j + 1],
                scalar2=None,
                op0=mybir.AluOpType.mult,
            )
            yield

    # ---- resumable filler generators (fine-grained: ~2 matmuls/item) ----
    def gen_xq(qc):
        emit_xs(qc)
        if qc == 1:
            # W_proj isn't needed until proj(0) runs mid-qc1; loading it
            # here keeps startup HBM bandwidth for the wq K/V phases
            nc.sync.dma_start(
                out=wp_sb, in_=wproj.rearrange("(i p) c -> p i c", p=128)
            )
            nc.sync.dma_start(out=wp3b, in_=wproj[3 * 128 + 64:4 * 128, :])
        xs_tiles = xs_store[qc]
        yield
        qts = []
        for p in range(HPC // 2):
            ps = ps_pool.tile([128, 512], F32, tag="ps", name=f"qps_{qc}_{p}")
            for c in range(NCT):
                nc.tensor.matmul(
                    ps, wq_sb[:, c, p * 128:(p + 1) * 128], xs_tiles[c],
                    start=(c == 0), stop=(c == NCT - 1),
                )
                if c % 2 == 1:
                    yield
            qt = qt_pool.tile([128, 512], MMD, tag="qt", name=f"qt_{qc}_{p}")
            nc.vector.tensor_copy(qt, ps)
            qts.append(qt)
            yield
        qts_store[qc] = qts

    def gen_kv(qc):
        """K^T tiles + V tiles for chunk qc, item order K0 V0 V1 V2 V3 K1
        K2 K3 (group granularity for the deadline counter; each group
        internally yields every 2 matmuls)."""
        xs_tiles = xs_store[qc]

        def k_item(p):
            ps = ps_pool.tile([128, 512], F32, tag="ps", name=f"kps_{qc}_{p}")
            for c in range(NCT):
                nc.tensor.matmul(
                    ps, wq_sb[:, c, FPC + p * 128:FPC + (p + 1) * 128],
                    xs_tiles[c],
                    start=(c == 0), stop=(c == NCT - 1),
                )
                if c % 2 == 1 and c < NCT - 1:
                    yield
            nc.vector.tensor_copy(kt[:, p, qc * 512:(qc + 1) * 512], ps)

        def v_item(j):
            kt_i = qc * 4 + j
            ps = ps_pool.tile([128, 512], F32, tag="ps", name=f"vps_{qc}_{j}")
            for c in range(NCT):
                nc.tensor.matmul(
                    ps, xs_tiles[c][:, j * 128:(j + 1) * 128],
                    wq_sb[:, c, 2 * FPC:3 * FPC],
                    start=(c == 0), stop=(c == NCT - 1),
                )
                if c % 2 == 1 and c < NCT - 1:
                    yield
            nc.vector.tensor_scalar(
                out=v2[:, kt_i, :, 0:HD],
                in0=ps.rearrange("p (h d) -> p h d", h=HPC),
                scalar1=eacol[:, kt_i:kt_i + 1],
                scalar2=None,
                op0=mybir.AluOpType.mult,
            )

        yield from k_item(0)
        yield  # group boundary: K0 complete
        for j in range(4):
            yield from v_item(j)
            yield  # Vj complete
        for p in range(1, HPC // 2):
            yield from k_item(p)
            yield  # Kp complete

    OUT_QUEUES = [None, None]

    def gen_proj(qc, stacks, pair_sel=None, target=None, row_base=None,
                 queues=None):
        tgt = out_ap if target is None else target
        rb = qc * 512 if row_base is None else row_base
        pr = list(range(HPC // 2) if pair_sel is None else pair_sel)
        qs = OUT_QUEUES if queues is None else queues
        qi = 0
        for tq in range(4):
            row0 = rb + tq * 128
            for ch in range(2):
                ps = ps_pool.tile([128, 512], F32, tag="ps",
                                  name=f"pps_{qc}_{tq}_{ch}")
                for i, p in enumerate(pr):
                    nc.tensor.matmul(
                        ps, stacks[p][:, tq * 128:(tq + 1) * 128],
                        wp_sb[:, p, ch * 512:(ch + 1) * 512],
                        start=(i == 0), stop=(i == len(pr) - 1),
                    )
                    if i % 2 == 1 and i < len(pr) - 1:
                        yield
                pout = pout_pool.tile([128, 512], MMD, tag="pout",
                                      name=f"po_{qc}_{tq}_{ch}")
                nc.vector.tensor_copy(pout, ps)
                q = qs[qi % len(qs)]
                qi += 1
                q.dma_start(
                    out=tgt[row0:row0 + 128, ch * 512:(ch + 1) * 512],
                    in_=pout,
                )
                yield

    OUT_QUEUES[0] = nc.sync
    OUT_QUEUES[1] = nc.gpsimd

    # Pre-emission of the next pair's first QK + exp at the previous
    # pair's end: fills the otherwise-serial QK->exp->PV pair-start
    # bubble on both the PE and ACT streams.
    pre_emitted = {}

    def emit_qk_exp(qc_, p_):
        Exp_ = mybir.ActivationFunctionType.Exp
        qt = qts_store[qc_][p_]
        st = st_pool.tile([128, 1024], F32, tag="st", name=f"st_{qc_}_{p_}_0")
        st3 = st.rearrange("p (h q) -> p h q", h=2)
        nc.tensor.matmul(
            st3[:, 0, :], kt[0:64, p_, 0:128], qt[0:64, :],
            start=True, stop=True,
        )
        nc.tensor.matmul(
            st3[:, 1, :], kt[64:128, p_, 0:128], qt[64:128, :],
            start=True, stop=True,
        )
        se = se_pool.tile([128, 1024], MMD, tag="se", name=f"se_{qc_}_{p_}_0")
        se3 = se.rearrange("p (h q) -> p h q", h=2)
        nc.scalar.activation(se3[:, 0, :], st3[:, 0, :], Exp_, scale=SCALE)
        nc.scalar.activation(se3[:, 1, :], st3[:, 1, :], Exp_, scale=SCALE)
        return (st3, se3)

    proj_gens = {}
    stacks_store = {}
    carry = []  # proj generators carried across chunk boundaries
    # The DRAM den-bounce finish (recip/scatter/broadcast/normalize) for
    # pair p is deferred to the NEXT pair's ki==fire_ki so the DVE never
    # queues behind the bounce's DMA latency.
    pending_den = [None]
    # K/V generation for chunks 2-3 flows across chunk boundaries: it is
    # pulled opportunistically wherever the PE has slack, with a deadline
    # force just before attention reaches the new key tiles.
    kv_gens = {}
    kv_pulled = {}

    def kv_force(c, n):
        g = kv_gens.get(c)
        while g is not None and kv_pulled[c] < n:
            try:
                next(g)
            except StopIteration:
                break
            kv_pulled[c] += 1

    def kv_filler(c):
        g = kv_gens[c]
        while True:
            try:
                next(g)
            except StopIteration:
                return
            kv_pulled[c] += 1
            yield

    for qc in range(NQC):
        fillers = []
        must_drain = []

        if qc == 0:
            kv_gens[0] = gen_v0rest()
            kv_pulled[0] = 0
            kv_gens[1] = gen_kv(1)
            kv_pulled[1] = 0
            kvf0 = kv_filler(0)
            g1 = gen_xq(1)
            fillers += [kvf0, g1, kv_filler(1)]
            must_drain += [kvf0, g1]
        elif qc == 1:
            kv_gens[2] = gen_kv(2)
            kv_pulled[2] = 0
            kvf1 = kv_filler(1)
            g1 = gen_xq(2)
            fillers += [kvf1, g1] + carry + [proj_gens.pop(0), kv_filler(2)]
            must_drain += [kvf1, g1]
            carry = []
        elif qc == 2:
            kv_gens[3] = gen_kv(3)
            kv_pulled[3] = 0
            kvf = kv_filler(2)
            g1 = gen_xq(3)
            fillers += [kvf, g1] + carry + [kv_filler(3)]
            must_drain += [kvf, g1] + carry  # carried proj(0) must finish
            carry = []
        else:
            kvf = kv_filler(3)
            fillers += [kvf] + carry + [proj_gens.pop(1)]
            must_drain += [kvf]
            carry = []

        def pull(n):
            for _ in range(n):
                while fillers:
                    try:
                        next(fillers[0])
                        break
                    except StopIteration:
                        fillers.pop(0)
                else:
                    break

        def drain_required():
            for g in must_drain:
                for _ in g:
                    pass

        # ---- attention for this query chunk, per head pair ----
        stacks = []
        nki = 4 * qc + 4
        # fine-grained items (~2 matmuls each): pull rate tuned so real
        # filler work lasts through the late (ACT-bound) pairs
        ppki = {0: 3, 1: 1, 2: 1, 3: 1}[qc]
        fire_ki = min(4, nki - 1)
        proj3c_gen = None
        qts = qts_store[qc]
        for p in range(HPC // 2):
            qt = qts[p]
            yA = y_pool.tile([128, 512], F32, tag="y", name=f"yA_{qc}_{p}")
            yB = y_pool.tile([128, 512], F32, tag="y", name=f"yB_{qc}_{p}")
            for ki in range(nki):
                if ki == fire_ki and pending_den[0] is not None:
                    pending_den[0]()
                    pending_den[0] = None
                if qc == NQC - 1 and p == 3 and ki == fire_ki + 1 \
                        and proj3c_gen is not None:
                    fillers.append(proj3c_gen)
                    proj3c_gen = None
                r = ki - 4 * qc
                if r >= 0 and qc in kv_gens:
                    if qc == 0:
                        kv_force(0, (r * 4) if p == 0 else 12)
                    else:
                        need_groups = (2 + r) if p == 0 else (5 + p)
                        kv_force(qc, need_groups * 4)  # 4 items per group
                n0 = 128 * r if r > 0 else 0
                if ki == 0 and (qc, p) in pre_emitted:
                    st3, se3 = pre_emitted.pop((qc, p))
                else:
                    st = st_pool.tile([128, 1024], F32, tag="st",
                                      name=f"st_{qc}_{p}_{ki}")
                    st3 = st.rearrange("p (h q) -> p h q", h=2)
                    kslice = slice(ki * 128, (ki + 1) * 128)
                    nc.tensor.matmul(
                        st3[:, 0, n0:512], kt[0:64, p, kslice], qt[0:64, n0:512],
                        start=True, stop=True,
                    )
                    nc.tensor.matmul(
                        st3[:, 1, n0:512], kt[64:128, p, kslice], qt[64:128, n0:512],
                        start=True, stop=True,
                    )
                    se = se_pool.tile([128, 1024], MMD, tag="se",
                                      name=f"se_{qc}_{p}_{ki}")
                    se3 = se.rearrange("p (h q) -> p h q", h=2)
                    if ki == 0:
                        nc.scalar.activation(
                            se3[:, 0, n0:512], st3[:, 0, n0:512], Exp, scale=SCALE
                        )
                        nc.scalar.activation(
                            se3[:, 1, n0:512], st3[:, 1, n0:512], Exp, scale=SCALE
                        )
                    else:
                        nc.scalar.activation(
                            se3[:, :, n0:512], st3[:, :, n0:512], Exp, scale=SCALE
                        )
                if r >= 0:
                    nc.vector.tensor_mul(
                        se3[:, 0, n0:n0 + 128], se3[:, 0, n0:n0 + 128], tri
                    )
                    nc.vector.tensor_mul(
                        se3[:, 1, n0:n0 + 128], se3[:, 1, n0:n0 + 128], tri
                    )
                nc.tensor.matmul(
                    yA[0:HD + 1, n0:512], v2[:, ki, 2 * p, :], se3[:, 0, n0:512],
                    start=(ki == 0), stop=(ki == nki - 1), skip_group_check=True,
                )
                nc.tensor.matmul(
                    yB[0:HD + 1, n0:512], v2[:, ki, 2 * p + 1, :], se3[:, 1, n0:512],
                    start=(ki == 0), stop=(ki == nki - 1), skip_group_check=True,
                )
                if qc == NQC - 1 and p <= 1:
                    # half-rate pulls early in qc3 so real filler work
                    # survives into the late (otherwise-starved) pairs
                    pull(1 if ki % 2 == 0 else 0)
                elif qc == NQC - 1 and p == 3:
                    pull(ppki + 1)
                else:
                    pull(ppki)

            # pre-emit the next pair's first QK + exps so its pipeline is
            # already primed while this pair evacuates/normalizes
            if p + 1 < HPC // 2:
                pre_emitted[(qc, p + 1)] = emit_qk_exp(qc, p + 1)

            if qc == NQC - 1 and p == HPC // 2 - 1:
                # Last pair: no stack at all.  The tail projects the
                # UNNORMALIZED per-head y (tmpA/tmpB recast bf16 below) and
                # folds 1/den into the evacuation as a per-partition scale:
                # the K=1 "spread" matmuls put den[q-tile j] on the
                # partitions as column j of dps, so rcp8[:, j] is exactly
                # the per-partition scale for q-tile j.  Heads A/B go to
                # separate partial outputs (summed on host).
                tmpA8 = tmp_pool.tile([128, 512], MMD, tag="stkB", bufs=3,
                                      name="tmpA8")
                nc.vector.tensor_copy(tmpA8[0:HD + 1, :], yA[0:HD + 1, :])
                tmpB8 = tmp_pool.tile([128, 512], MMD, tag="stkB", bufs=3,
                                      name="tmpB8")
                nc.vector.tensor_copy(tmpB8[0:HD + 1, :], yB[0:HD + 1, :])
                dps = y_pool.tile([128, 8], F32, tag="y", name="dps")
                for j in range(4):
                    nc.tensor.matmul(
                        dps[:, j:j + 1],
                        tmpA8[HD:HD + 1, j * 128:(j + 1) * 128],
                        oneb[HD:HD + 1, :],
                        start=True, stop=True, skip_group_check=True,
                    )
                    nc.tensor.matmul(
                        dps[:, 4 + j:5 + j],
                        tmpB8[HD:HD + 1, j * 128:(j + 1) * 128],
                        oneb[HD:HD + 1, :],
                        start=True, stop=True, skip_group_check=True,
                    )
                rcp8 = rec_pool.tile([128, 8], F32, tag="rcp", name="rcp_last")
                nc.vector.reciprocal(rcp8, dps)
                last_pair = (tmpA8, tmpB8, rcp8)
                stack = None
            else:
                # evacuate y^T + denominators; den rows bounce through
                # DRAM now, but the finish (recip/scatter/broadcast/
                # normalize) is deferred to the next pair's ki==fire_ki so
                # the DVE never queues behind the bounce latency.
                tmpA = tmp_pool.tile([128, 512], F32, tag="tmp",
                                     name=f"tmpA_{qc}_{p}")
                nc.vector.tensor_copy(tmpA[0:HD + 1, :], yA[0:HD + 1, :])
                tmpB = tmp_pool.tile([128, 512], F32, tag="tmp",
                                     name=f"tmpB_{qc}_{p}")
                nc.vector.tensor_copy(tmpB[0:HD + 1, :], yB[0:HD + 1, :])
                idx = qc * 4 + p
                dq = nc.scalar if qc <= 1 else nc.gpsimd
                dq.dma_start(out=dsc1[idx, 0:512], in_=tmpA[HD:HD + 1, :])
                nc.sync.dma_start(out=dsc1[idx, 512:1024], in_=tmpB[HD:HD + 1, :])
                dnp = rec_pool.tile([128, 8], F32, tag="dnp", name=f"dnp_{qc}_{p}")
                dq.dma_start(
                    out=dnp, in_=dsc1[idx, :].rearrange("(p j) -> p j", p=128)
                )
                rcp = rec_pool.tile([128, 8], F32, tag="rcp", name=f"rcp_{qc}_{p}")
                sc = scale_pool.tile([64, 1024], F32, tag="scale",
                                     name=f"sc_{qc}_{p}")
                stack = stack_pool.tile([128, 512], MMD, tag="stack",
                                        name=f"stk_{qc}_{p}")
                stkB = tmp_pool.tile([64, 512], MMD, tag="stkB", bufs=3,
                                     name=f"skB_{qc}_{p}")

                def den_finish(idx=idx, dq=dq, dnp=dnp, rcp=rcp, sc=sc,
                               stack=stack, stkB=stkB, tmpA=tmpA, tmpB=tmpB):
                    nc.vector.reciprocal(rcp, dnp)
                    dq.dma_start(
                        out=dsc2[idx, :].rearrange("(p j) -> p j", p=128),
                        in_=rcp,
                    )
                    dq.dma_start(
                        out=sc[0:64, :],
                        in_=dsc2[idx:idx + 1, :].to_broadcast([64, 1024]),
                    )
                    nc.vector.tensor_mul(stack[0:64, :], tmpA[0:64, :],
                                         sc[0:64, 0:512])
                    nc.vector.tensor_mul(stkB[0:64, :], tmpB[0:64, :],
                                         sc[0:64, 512:1024])
                    dq.dma_start(out=stack[64:128, :], in_=stkB[0:64, :])

                pending_den[0] = den_finish
            stacks.append(stack)

            if qc == NQC - 1 and p == 2:
                fillers.append(gen_proj(qc, stacks, pair_sel=[0, 1]))
                fillers.append(proj_gens.pop(2))
                proj3c_gen = gen_proj(qc, stacks, pair_sel=[2],
                                      target=out2_ap, row_base=0)
            pull(2)

        if qc in kv_gens:
            kv_force(qc, 1000)
        drain_required()
        if qc + 1 < NQC:
            # qts for the next chunk exist now (gen_xq drained above)
            pre_emitted[(qc + 1, 0)] = emit_qk_exp(qc + 1, 0)
        stacks_store[qc] = stacks
        if qc < NQC - 1:
            proj_gens[qc] = gen_proj(qc, stacks)
        carry = [g for g in fillers if g not in must_drain]

    # drain anything still pending (proj3a / proj3c leftovers)
    for g in carry:
        for _ in g:
            pass

    # ---- tail: pair 3 of the final chunk into out3, as K=64 matmuls of
    # the UNNORMALIZED per-head y with 1/den folded into the evacuation
    # (per-partition scale from rcp8): halfA = psA*rcpA on the ACT engine,
    # pout = psB*rcpB + halfA fused on the DVE. ----
    tmpA8, tmpB8, rcp8 = last_pair
    tail_queues = [nc.sync, nc.scalar]
    slot_iter = [(ps_pool, "ps"), (st_pool, "st"), (y_pool, "y")]
    units = [(tq, ch) for tq in range(4) for ch in range(2)]
    for i, (tq, ch) in enumerate(units):
        poolA, tagA = slot_iter[(2 * i) % 3]
        poolB, tagB = slot_iter[(2 * i + 1) % 3]
        psA = poolA.tile([128, 512], F32, tag=tagA, name=f"t3a_{tq}_{ch}")
        nc.tensor.matmul(
            psA, tmpA8[0:64, tq * 128:(tq + 1) * 128],
            wp_sb[0:64, 3, ch * 512:(ch + 1) * 512],
            start=True, stop=True, skip_group_check=True,
        )
        psB = poolB.tile([128, 512], F32, tag=tagB, name=f"t3b_{tq}_{ch}")
        nc.tensor.matmul(
            psB, tmpB8[0:64, tq * 128:(tq + 1) * 128],
            wp3b[0:64, ch * 512:(ch + 1) * 512],
            start=True, stop=True, skip_group_check=True,
        )
        halfA = tmp_pool.tile([128, 512], F32, tag="tmp", name=f"ha_{tq}_{ch}")
        nc.scalar.activation(halfA, psA, Copy, scale=rcp8[:, tq:tq + 1])
        pout = pout_pool.tile([128, 512], MMD, tag="pout",
                              name=f"po3_{tq}_{ch}")
        nc.vector.scalar_tensor_tensor(
            out=pout, in0=psB, scalar=rcp8[:, 4 + tq:5 + tq], in1=halfA,
            op0=mybir.AluOpType.mult, op1=mybir.AluOpType.add,
        )
        tail_queues[i % 2].dma_start(
            out=out3_ap[tq * 128:(tq + 1) * 128, ch * 512:(ch + 1) * 512],
            in_=pout,
        )


def make_nc():
    nc = bacc.Bacc("TRN2", target_bir_lowering=False, debug=False,
                   num_devices=NCORES)
    xT = nc.dram_tensor("xT", [C, T], MMD, kind="ExternalInput")
    wqkv = nc.dram_tensor("wqkv", [C, 3 * FPC], MMD, kind="ExternalInput")
    wproj = nc.dram_tensor("wproj", [FPC, C], MMD, kind="ExternalInput")
    ea = nc.dram_tensor("ea", [T], F32, kind="ExternalInput")
    out = nc.dram_tensor("out", [T, C], MMD, kind="ExternalOutput")
    out2 = nc.dram_tensor("out2", [512, C], MMD, kind="ExternalOutput")
    out3 = nc.dram_tensor("out3", [512, C], MMD, kind="ExternalOutput")
    dsc1 = nc.dram_tensor("dsc1", [16, 1024], F32, kind="Internal")
    dsc2 = nc.dram_tensor("dsc2", [16, 1024], F32, kind="Internal")
    tri_np = np.triu(np.ones((128, 128), dtype=NPMMD))
    tri_dram = nc.inline_tensor(tri_np, name="tri_const")
    with ExitStack() as ctx:
        tc = ctx.enter_context(tile.TileContext(nc))
        tc.ctx = ctx
        build(tc, out[:, :], out2[:, :], out3[:, :], xT[:, :],
              wqkv[:, :], wproj[:, :], ea[:], tri_dram, dsc1[:, :],
              dsc2[:, :])
    nc.compile()
    return nc


def shard_inputs(x, prev_probs, W_attn, W_proj):
    in_maps = []
    for core in range(NCORES):
        b, g = divmod(core, 2)
        xT = np.ascontiguousarray(x[b].T)
        wq = W_attn[:, g * FPC:(g + 1) * FPC]
        wk = W_attn[:, C + g * FPC:C + (g + 1) * FPC]
        wv = W_attn[:, 2 * C + g * FPC:2 * C + (g + 1) * FPC]
        wqkv = np.ascontiguousarray(np.concatenate([wq, wk, wv], axis=1))
        wproj = np.ascontiguousarray(W_proj[g * FPC:(g + 1) * FPC, :])
        ea = np.power(prev_probs[b] + np.float32(1e-10), np.float32(-EPS_BIAS))
        in_maps.append(
            {
                "xT": xT.astype(NPMMD),
                "wqkv": wqkv.astype(NPMMD),
                "wproj": wproj.astype(NPMMD),
                "ea": ea.astype(np.float32),
            }
        )
    return in_maps


_CACHED_NC = None


def kernel(x, prev_probs, W_attn, W_proj, trace=False, tmpdir=None):
    global _CACHED_NC
    from concourse.bass_utils import run_bass_kernel_spmd

    x = np.asarray(x, dtype=np.float32)
    prev_probs = np.asarray(prev_probs, dtype=np.float32)
    W_attn = np.asarray(W_attn, dtype=np.float32)
    W_proj = np.asarray(W_proj, dtype=np.float32)

    if _CACHED_NC is None:
        _CACHED_NC = make_nc()
    nc = _CACHED_NC

    in_maps = shard_inputs(x, prev_probs, W_attn, W_proj)
    res = run_bass_kernel_spmd(
        nc, in_maps, core_ids=list(range(NCORES)), trace=trace, tmpdir=tmpdir
    )
    out = np.empty((B, T, C), dtype=np.float32)
    for b in range(B):
        acc = (res.results[2 * b]["out"].astype(np.float32)
               + res.results[2 * b + 1]["out"].astype(np.float32))
        for part in ("out2", "out3"):
            acc[3 * 512:] += (res.results[2 * b][part].astype(np.float32)
                              + res.results[2 * b + 1][part].astype(np.float32))
        out[b] = acc
    kernel.last_results = res
    return out


# revision 51
# speedup vs baseline: 1.0583x; 1.0583x over previous
"""Trainium2 Bass kernel for CausalSelfAttention (B=4, T=2048, C=1024, H=16)
with additive prev-prob key bias.

Sharding: 8 cores = data-parallel over B (4) x tensor-parallel over head
halves (2).  Each core computes qkv for its 8 heads, causal attention, and a
partial output projection (row-parallel W_proj); host sums the partials
per batch at unshard time.  Each core emits three partial tensors: `out`
(all rows; final-chunk rows carry only pairs 0-1), `out2` (final-chunk
rows, pair 2) and `out3` (final-chunk rows, pair 3).

Per-core device algorithm (v3 — phase-balanced schedule):
  - All matmul operands are bf16 (PSUM accumulation stays fp32).
  - K^T and Q^T kept feature-major with head pairs stacked in the 128
    partitions, so QK^T runs as two K=64 matmuls co-issued on row-group
    halves — full-array throughput despite K=64.
  - Scores are computed transposed (keys on partitions): softmax denominator
    comes from an extra EA column appended to V (M=65 PV matmuls), where
    EA[k] = (prev_probs[k]+1e-10)**-EPS folds the additive log bias into a
    multiplicative per-key scale of exp(qk/8).
  - Causality: block-trimmed matmul widths + one 128x128 triangular mask
    multiply (bf16, DVE 2x rate) per diagonal block.
  - PHASE BALANCE: the scalar/ACT engine (exp) is the pole in late query
    chunks (qc=3 needs ~64us of exp vs ~37us of attention matmul), while
    early chunks are PE-bound.  PE work is deferred into late chunks:
    K/V generation for chunks 2-3 is pulled into those chunks' own
    attention (deadline-forced just before ki crosses into the new key
    tiles), proj(1)/proj(2) run during qc=3, and the final chunk's
    projection is split per pair group: pairs 0-1 mid-qc3 into `out`,
    pair 2 into `out2` while pair 3's attention runs, pair 3 in the tail
    into `out3` as K=64 matmuls (so the B-half never needs the partition-
    shift DMA).  Filler generators yield every ~2 matmuls so the exp
    stream is never starved behind a coarse filler burst, and dry pulls
    emit dependency-free warm matmuls to keep the HAM clock at 2.4GHz.
  - Startup: weights are DMAed in Q|K|V column phases and chunk-0 QKV
    runs c-major across 4 concurrent PSUM groups so the PE tracks DMA
    arrival densely.
  - 1/denominator for most pairs: bounce the den row through DRAM on the
    gpsimd SWDGE queue; the dependent stack multiplies also run on gpsimd
    so the ~10us latency never blocks the DVE.  The very last pair
    computes 1/den on-chip (K=1 spread matmuls + PE transpose + rank-1
    broadcast) so the tail isn't gated by DMA round trips.
"""

import math
from contextlib import ExitStack

import ml_dtypes
import numpy as np

import concourse.bass as bass
import concourse.tile as tile
from concourse import bacc, mybir

F32 = mybir.dt.float32
MMD = mybir.dt.bfloat16
NPMMD = ml_dtypes.bfloat16

B, T, C, H = 4, 2048, 1024, 16
HD = C // H          # 64
NCORES = 8
HPC = H // 2         # 8 heads per core
FPC = HPC * HD       # 512 features per core
NKT = T // 128       # 16 key tiles
NQC = T // 512       # 4 query chunks (also the x t-chunks)
NCT = C // 128       # 8 contraction tiles
EPS_BIAS = 0.1
SCALE = 1.0 / math.sqrt(HD)


def build(tc, out_ap, out2_ap, out3_ap, xT, wqkv, wproj, ea,
          tri_dram, dsc1, dsc2):
    """Emit the per-core kernel into TileContext tc."""
    nc = tc.nc
    ctx = tc.ctx
    Exp = mybir.ActivationFunctionType.Exp
    Copy = mybir.ActivationFunctionType.Copy

    const = ctx.enter_context(tc.tile_pool(name="const", bufs=1))
    xs_pool = ctx.enter_context(tc.tile_pool(name="xs", bufs=16))
    qt_pool = ctx.enter_context(tc.tile_pool(name="qt", bufs=5))
    se_pool = ctx.enter_context(tc.tile_pool(name="se", bufs=4))
    tmp_pool = ctx.enter_context(tc.tile_pool(name="tmp", bufs=6))
    rec_pool = ctx.enter_context(tc.tile_pool(name="rec", bufs=4))
    scale_pool = ctx.enter_context(tc.tile_pool(name="scale", bufs=4))
    stack_pool = ctx.enter_context(tc.tile_pool(name="stack", bufs=12))
    pout_pool = ctx.enter_context(tc.tile_pool(name="pout", bufs=6))

    ps_pool = ctx.enter_context(tc.tile_pool(name="ps", bufs=2, space="PSUM"))
    st_pool = ctx.enter_context(tc.tile_pool(name="st", bufs=2, space="PSUM"))
    y_pool = ctx.enter_context(tc.tile_pool(name="y", bufs=2, space="PSUM"))

    # ---- constants / persistent buffers ----
    tri = const.tile([128, 128], MMD, name="tri")
    eacol = const.tile([128, NKT], F32, name="eacol")
    wp_sb = const.tile([128, FPC // 128, C], MMD, name="wp_sb")     # 8KB/p
    wp3b = const.tile([64, C], MMD, name="wp3b")  # W_proj rows 448:512 at base 0

    # small consts ride the scalar hw queue so the sync queue starts on
    # chunk-0 x tiles immediately
    nc.scalar.dma_start(out=eacol, in_=ea.rearrange("(k p) -> p k", p=128))
    nc.scalar.dma_start(out=tri, in_=tri_dram[:, :])

    oneb = const.tile([128, 1], MMD, name="oneb")
    nc.vector.memset(oneb, 1.0)

    wq_sb = const.tile([128, NCT, 3 * FPC], MMD, name="wq_sb")      # 24KB/p
    wqkv3 = wqkv.rearrange("(c p) f -> p c f", p=128)
    for part in range(3):  # Q columns, then K, then V
        for c in range(NCT):
            nc.gpsimd.dma_start(
                out=wq_sb[:, c, part * FPC:(part + 1) * FPC],
                in_=wqkv3[:, c, part * FPC:(part + 1) * FPC],
            )

    kt = const.tile([128, HPC // 2, T], MMD, name="kt")             # 16KB/p
    v2 = const.tile([128, NKT, HPC, HD + 1], MMD, name="v2")        # 16.6KB/p
    ones8 = const.tile([128, HPC], F32, name="ones8")
    nc.vector.memset(ones8, 1.0)
    warm = const.tile([128, 512], MMD, name="warm")
    nc.vector.memset(warm, 0.5)

    for kt_i in range(NKT):
        nc.vector.tensor_scalar(
            out=v2[:, kt_i, :, HD:HD + 1],
            in0=ones8.unsqueeze(2),
            scalar1=eacol[:, kt_i:kt_i + 1],
            scalar2=None,
            op0=mybir.AluOpType.mult,
        )

    qts_store = {}
    xs_store = {}

    def emit_warm(n=1):
        wps = ps_pool.tile([128, 512], F32, tag="ps", name="warmps")
        for _ in range(n):
            nc.tensor.matmul(wps, warm[:, 0:128], warm,
                             start=True, stop=True, skip_group_check=True)

    def emit_xs(qc):
        xs_tiles = []
        for c in range(NCT):
            xs = xs_pool.tile([128, 512], MMD, tag="xs", name=f"xs_{qc}_{c}")
            nc.sync.dma_start(
                out=xs, in_=xT[c * 128:(c + 1) * 128, qc * 512:(qc + 1) * 512]
            )
            xs_tiles.append(xs)
        xs_store[qc] = xs_tiles

    # ---- chunk 0 QKV: fully c-major across 4 concurrent PSUM groups so
    # the PE tracks the weight-phase DMA arrival densely. ----
    emit_xs(0)

    # dep-free warm burst: trips the HAM activity monitor to full clock
    # while the first weight/x tiles land
    emit_warm(8)

    xs0 = xs_store[0]

    def four_slots(pref):
        return [
            ps_pool.tile([128, 512], F32, tag="ps", name=f"{pref}_0"),
            ps_pool.tile([128, 512], F32, tag="ps", name=f"{pref}_1"),
            st_pool.tile([128, 512], F32, tag="st", name=f"{pref}_2"),
            y_pool.tile([128, 512], F32, tag="y", name=f"{pref}_3"),
        ]

    qps = four_slots("q0")
    for c in range(NCT):
        for p in range(HPC // 2):
            nc.tensor.matmul(
                qps[p], wq_sb[:, c, p * 128:(p + 1) * 128], xs0[c],
                start=(c == 0), stop=(c == NCT - 1),
            )
    qts0 = []
    for p in range(HPC // 2):
        qt = qt_pool.tile([128, 512], MMD, tag="qt", name=f"qt_0_{p}")
        nc.vector.tensor_copy(qt, qps[p])
        qts0.append(qt)
    qts_store[0] = qts0

    kps = four_slots("k0")
    for c in range(NCT):
        for p in range(HPC // 2):
            nc.tensor.matmul(
                kps[p], wq_sb[:, c, FPC + p * 128:FPC + (p + 1) * 128], xs0[c],
                start=(c == 0), stop=(c == NCT - 1),
            )
    for p in range(HPC // 2):
        nc.vector.tensor_copy(kt[:, p, 0:512], kps[p])

    # V0 only — V1..V3 are deferred into qc0's attention as deadline-
    # forced fillers so the first exps start ~5us earlier
    vps0 = ps_pool.tile([128, 512], F32, tag="ps", name="v0_0")
    for c in range(NCT):
        nc.tensor.matmul(
            vps0, xs0[c][:, 0:128], wq_sb[:, c, 2 * FPC:3 * FPC],
            start=(c == 0), stop=(c == NCT - 1),
        )
    nc.vector.tensor_scalar(
        out=v2[:, 0, :, 0:HD],
        in0=vps0.rearrange("p (h d) -> p h d", h=HPC),
        scalar1=eacol[:, 0:1],
        scalar2=None,
        op0=mybir.AluOpType.mult,
    )

    def gen_v0rest():
        for j in range(1, 4):
            ps = ps_pool.tile([128, 512], F32, tag="ps", name=f"vps_0_{j}")
            for c in range(NCT):
                nc.tensor.matmul(
                    ps, xs0[c][:, j * 128:(j + 1) * 128],
                    wq_sb[:, c, 2 * FPC:3 * FPC],
                    start=(c == 0), stop=(c == NCT - 1),
                )
                if c % 2 == 1 and c < NCT - 1:
                    yield
            nc.vector.tensor_scalar(
                out=v2[:, j, :, 0:HD],
                in0=ps.rearrange("p (h d) -> p h d", h=HPC),
                scalar1=eacol[:, j:j + 1],
                scalar2=None,
                op0=mybir.AluOpType.mult,
            )
            yield

    # ---- resumable filler generators (fine-grained: ~2 matmuls/item) ----
    def gen_xq(qc):
        emit_xs(qc)
        if qc == 1:
            # W_proj isn't needed until proj(0) runs mid-qc1; loading it
            # here keeps startup HBM bandwidth for the wq K/V phases
            nc.sync.dma_start(
                out=wp_sb, in_=wproj.rearrange("(i p) c -> p i c", p=128)
            )
            nc.sync.dma_start(out=wp3b, in_=wproj[3 * 128 + 64:4 * 128, :])
        xs_tiles = xs_store[qc]
        yield
        qts = []
        for p in range(HPC // 2):
            ps = ps_pool.tile([128, 512], F32, tag="ps", name=f"qps_{qc}_{p}")
            for c in range(NCT):
                nc.tensor.matmul(
                    ps, wq_sb[:, c, p * 128:(p + 1) * 128], xs_tiles[c],
                    start=(c == 0), stop=(c == NCT - 1),
                )
                if c % 2 == 1:
                    yield
            qt = qt_pool.tile([128, 512], MMD, tag="qt", name=f"qt_{qc}_{p}")
            nc.vector.tensor_copy(qt, ps)
            qts.append(qt)
            yield
        qts_store[qc] = qts

    def gen_kv(qc):
        """K^T tiles + V tiles for chunk qc, item order K0 V0 V1 V2 V3 K1
        K2 K3 (group granularity for the deadline counter; each group
        internally yields every 2 matmuls)."""
        xs_tiles = xs_store[qc]

        def k_item(p):
            ps = ps_pool.tile([128, 512], F32, tag="ps", name=f"kps_{qc}_{p}")
            for c in range(NCT):
                nc.tensor.matmul(
                    ps, wq_sb[:, c, FPC + p * 128:FPC + (p + 1) * 128],
                    xs_tiles[c],
                    start=(c == 0), stop=(c == NCT - 1),
                )
                if c % 2 == 1 and c < NCT - 1:
                    yield
            nc.vector.tensor_copy(kt[:, p, qc * 512:(qc + 1) * 512], ps)

        def v_item(j):
            kt_i = qc * 4 + j
            ps = ps_pool.tile([128, 512], F32, tag="ps", name=f"vps_{qc}_{j}")
            for c in range(NCT):
                nc.tensor.matmul(
                    ps, xs_tiles[c][:, j * 128:(j + 1) * 128],
                    wq_sb[:, c, 2 * FPC:3 * FPC],
                    start=(c == 0), stop=(c == NCT - 1),
                )
                if c % 2 == 1 and c < NCT - 1:
                    yield
            nc.vector.tensor_scalar(
                out=v2[:, kt_i, :, 0:HD],
                in0=ps.rearrange("p (h d) -> p h d", h=HPC),
                scalar1=eacol[:, kt_i:kt_i + 1],
                scalar2=None,
                op0=mybir.AluOpType.mult,
            )

        yield from k_item(0)
        yield  # group boundary: K0 complete
        for j in range(4):
            yield from v_item(j)
            yield  # Vj complete
        for p in range(1, HPC // 2):
            yield from k_item(p)
            yield  # Kp complete

    OUT_QUEUES = [None, None]

    def gen_proj(qc, stacks, pair_sel=None, target=None, row_base=None,
                 queues=None):
        tgt = out_ap if target is None else target
        rb = qc * 512 if row_base is None else row_base
        pr = list(range(HPC // 2) if pair_sel is None else pair_sel)
        qs = OUT_QUEUES if queues is None else queues
        qi = 0
        for tq in range(4):
            row0 = rb + tq * 128
            for ch in range(2):
                ps = ps_pool.tile([128, 512], F32, tag="ps",
                                  name=f"pps_{qc}_{tq}_{ch}")
                for i, p in enumerate(pr):
                    nc.tensor.matmul(
                        ps, stacks[p][:, tq * 128:(tq + 1) * 128],
                        wp_sb[:, p, ch * 512:(ch + 1) * 512],
                        start=(i == 0), stop=(i == len(pr) - 1),
                    )
                    if i % 2 == 1 and i < len(pr) - 1:
                        yield
                pout = pout_pool.tile([128, 512], MMD, tag="pout",
                                      name=f"po_{qc}_{tq}_{ch}")
                nc.vector.tensor_copy(pout, ps)
                q = qs[qi % len(qs)]
                qi += 1
                q.dma_start(
                    out=tgt[row0:row0 + 128, ch * 512:(ch + 1) * 512],
                    in_=pout,
                )
                yield

    OUT_QUEUES[0] = nc.sync
    OUT_QUEUES[1] = nc.gpsimd

    # Pre-emission of the next pair's first QK + exp at the previous
    # pair's end: fills the otherwise-serial QK->exp->PV pair-start
    # bubble on both the PE and ACT streams.
    pre_emitted = {}

    def emit_qk_exp(qc_, p_):
        Exp_ = mybir.ActivationFunctionType.Exp
        qt = qts_store[qc_][p_]
        st = st_pool.tile([128, 1024], F32, tag="st", name=f"st_{qc_}_{p_}_0")
        st3 = st.rearrange("p (h q) -> p h q", h=2)
        nc.tensor.matmul(
            st3[:, 0, :], kt[0:64, p_, 0:128], qt[0:64, :],
            start=True, stop=True,
        )
        nc.tensor.matmul(
            st3[:, 1, :], kt[64:128, p_, 0:128], qt[64:128, :],
            start=True, stop=True,
        )
        se = se_pool.tile([128, 1024], MMD, tag="se", name=f"se_{qc_}_{p_}_0")
        se3 = se.rearrange("p (h q) -> p h q", h=2)
        nc.scalar.activation(se3[:, 0, :], st3[:, 0, :], Exp_, scale=SCALE)
        nc.scalar.activation(se3[:, 1, :], st3[:, 1, :], Exp_, scale=SCALE)
        return (st3, se3)

    proj_gens = {}
    stacks_store = {}
    carry = []  # proj generators carried across chunk boundaries
    # The DRAM den-bounce finish (recip/scatter/broadcast/normalize) for
    # pair p is deferred to the NEXT pair's ki==fire_ki so the DVE never
    # queues behind the bounce's DMA latency.
    pending_den = [None]
    # K/V generation for chunks 2-3 flows across chunk boundaries: it is
    # pulled opportunistically wherever the PE has slack, with a deadline
    # force just before attention reaches the new key tiles.
    kv_gens = {}
    kv_pulled = {}

    def kv_force(c, n):
        g = kv_gens.get(c)
        while g is not None and kv_pulled[c] < n:
            try:
                next(g)
            except StopIteration:
                break
            kv_pulled[c] += 1

    def kv_filler(c):
        g = kv_gens[c]
        while True:
            try:
                next(g)
            except StopIteration:
                return
            kv_pulled[c] += 1
            yield

    for qc in range(NQC):
        fillers = []
        must_drain = []

        if qc == 0:
            kv_gens[0] = gen_v0rest()
            kv_pulled[0] = 0
            kvf0 = kv_filler(0)
            g1, g2 = gen_xq(1), gen_kv(1)
            fillers += [kvf0, g1, g2]
            must_drain += [kvf0, g1, g2]
        elif qc == 1:
            kv_gens[2] = gen_kv(2)
            kv_pulled[2] = 0
            g1 = gen_xq(2)
            fillers += [g1] + carry + [proj_gens.pop(0), kv_filler(2)]
            must_drain += [g1]
            carry = []
        elif qc == 2:
            kv_gens[3] = gen_kv(3)
            kv_pulled[3] = 0
            kvf = kv_filler(2)
            g1 = gen_xq(3)
            fillers += [kvf, g1] + carry + [kv_filler(3)]
            must_drain += [kvf, g1] + carry  # carried proj(0) must finish
            carry = []
        else:
            kvf = kv_filler(3)
            fillers += [kvf] + carry + [proj_gens.pop(1)]
            must_drain += [kvf]
            carry = []

        def pull(n):
            for _ in range(n):
                while fillers:
                    try:
                        next(fillers[0])
                        break
                    except StopIteration:
                        fillers.pop(0)
                else:
                    break

        def drain_required():
            for g in must_drain:
                for _ in g:
                    pass

        # ---- attention for this query chunk, per head pair ----
        stacks = []
        nki = 4 * qc + 4
        # fine-grained items (~2 matmuls each): pull rate tuned so real
        # filler work lasts through the late (ACT-bound) pairs
        ppki = {0: 3, 1: 1, 2: 1, 3: 1}[qc]
        fire_ki = min(4, nki - 1)
        proj3c_gen = None
        qts = qts_store[qc]
        for p in range(HPC // 2):
            qt = qts[p]
            yA = y_pool.tile([128, 512], F32, tag="y", name=f"yA_{qc}_{p}")
            yB = y_pool.tile([128, 512], F32, tag="y", name=f"yB_{qc}_{p}")
            for ki in range(nki):
                if ki == fire_ki and pending_den[0] is not None:
                    pending_den[0]()
                    pending_den[0] = None
                if qc == NQC - 1 and p == 3 and ki == fire_ki + 1 \
                        and proj3c_gen is not None:
                    fillers.append(proj3c_gen)
                    proj3c_gen = None
                r = ki - 4 * qc
                if r >= 0 and qc in kv_gens:
                    if qc == 0:
                        kv_force(0, (r * 4) if p == 0 else 12)
                    else:
                        need_groups = (2 + r) if p == 0 else (5 + p)
                        kv_force(qc, need_groups * 4)  # 4 items per group
                n0 = 128 * r if r > 0 else 0
                if ki == 0 and (qc, p) in pre_emitted:
                    st3, se3 = pre_emitted.pop((qc, p))
                else:
                    st = st_pool.tile([128, 1024], F32, tag="st",
                                      name=f"st_{qc}_{p}_{ki}")
                    st3 = st.rearrange("p (h q) -> p h q", h=2)
                    kslice = slice(ki * 128, (ki + 1) * 128)
                    nc.tensor.matmul(
                        st3[:, 0, n0:512], kt[0:64, p, kslice], qt[0:64, n0:512],
                        start=True, stop=True,
                    )
                    nc.tensor.matmul(
                        st3[:, 1, n0:512], kt[64:128, p, kslice], qt[64:128, n0:512],
                        start=True, stop=True,
                    )
                    se = se_pool.tile([128, 1024], MMD, tag="se",
                                      name=f"se_{qc}_{p}_{ki}")
                    se3 = se.rearrange("p (h q) -> p h q", h=2)
                    if ki == 0:
                        nc.scalar.activation(
                            se3[:, 0, n0:512], st3[:, 0, n0:512], Exp, scale=SCALE
                        )
                        nc.scalar.activation(
                            se3[:, 1, n0:512], st3[:, 1, n0:512], Exp, scale=SCALE
                        )
                    else:
                        nc.scalar.activation(
                            se3[:, :, n0:512], st3[:, :, n0:512], Exp, scale=SCALE
                        )
                if r >= 0:
                    nc.vector.tensor_mul(
                        se3[:, 0, n0:n0 + 128], se3[:, 0, n0:n0 + 128], tri
                    )
                    nc.vector.tensor_mul(
                        se3[:, 1, n0:n0 + 128], se3[:, 1, n0:n0 + 128], tri
                    )
                nc.tensor.matmul(
                    yA[0:HD + 1, n0:512], v2[:, ki, 2 * p, :], se3[:, 0, n0:512],
                    start=(ki == 0), stop=(ki == nki - 1), skip_group_check=True,
                )
                nc.tensor.matmul(
                    yB[0:HD + 1, n0:512], v2[:, ki, 2 * p + 1, :], se3[:, 1, n0:512],
                    start=(ki == 0), stop=(ki == nki - 1), skip_group_check=True,
                )
                if qc == NQC - 1 and p <= 1:
                    # half-rate pulls early in qc3 so real filler work
                    # survives into the late (otherwise-starved) pairs
                    pull(1 if ki % 2 == 0 else 0)
                elif qc == NQC - 1 and p == 3:
                    pull(ppki + 1)
                else:
                    pull(ppki)

            # pre-emit the next pair's first QK + exps so its pipeline is
            # already primed while this pair evacuates/normalizes
            if p + 1 < HPC // 2:
                pre_emitted[(qc, p + 1)] = emit_qk_exp(qc, p + 1)

            if qc == NQC - 1 and p == HPC // 2 - 1:
                # Last pair: no stack at all.  The tail projects the
                # UNNORMALIZED per-head y (tmpA/tmpB recast bf16 below) and
                # folds 1/den into the evacuation as a per-partition scale:
                # the K=1 "spread" matmuls put den[q-tile j] on the
                # partitions as column j of dps, so rcp8[:, j] is exactly
                # the per-partition scale for q-tile j.  Heads A/B go to
                # separate partial outputs (summed on host).
                tmpA8 = tmp_pool.tile([128, 512], MMD, tag="stkB", bufs=3,
                                      name="tmpA8")
                nc.vector.tensor_copy(tmpA8[0:HD + 1, :], yA[0:HD + 1, :])
                tmpB8 = tmp_pool.tile([128, 512], MMD, tag="stkB", bufs=3,
                                      name="tmpB8")
                nc.vector.tensor_copy(tmpB8[0:HD + 1, :], yB[0:HD + 1, :])
                dps = y_pool.tile([128, 8], F32, tag="y", name="dps")
                for j in range(4):
                    nc.tensor.matmul(
                        dps[:, j:j + 1],
                        tmpA8[HD:HD + 1, j * 128:(j + 1) * 128],
                        oneb[HD:HD + 1, :],
                        start=True, stop=True, skip_group_check=True,
                    )
                    nc.tensor.matmul(
                        dps[:, 4 + j:5 + j],
                        tmpB8[HD:HD + 1, j * 128:(j + 1) * 128],
                        oneb[HD:HD + 1, :],
                        start=True, stop=True, skip_group_check=True,
                    )
                rcp8 = rec_pool.tile([128, 8], F32, tag="rcp", name="rcp_last")
                nc.vector.reciprocal(rcp8, dps)
                last_pair = (tmpA8, tmpB8, rcp8)
                stack = None
            else:
                # evacuate y^T + denominators; den rows bounce through
                # DRAM now, but the finish (recip/scatter/broadcast/
                # normalize) is deferred to the next pair's ki==fire_ki so
                # the DVE never queues behind the bounce latency.
                tmpA = tmp_pool.tile([128, 512], F32, tag="tmp",
                                     name=f"tmpA_{qc}_{p}")
                nc.vector.tensor_copy(tmpA[0:HD + 1, :], yA[0:HD + 1, :])
                tmpB = tmp_pool.tile([128, 512], F32, tag="tmp",
                                     name=f"tmpB_{qc}_{p}")
                nc.vector.tensor_copy(tmpB[0:HD + 1, :], yB[0:HD + 1, :])
                idx = qc * 4 + p
                dq = nc.scalar if qc <= 1 else nc.gpsimd
                dq.dma_start(out=dsc1[idx, 0:512], in_=tmpA[HD:HD + 1, :])
                nc.sync.dma_start(out=dsc1[idx, 512:1024], in_=tmpB[HD:HD + 1, :])
                dnp = rec_pool.tile([128, 8], F32, tag="dnp", name=f"dnp_{qc}_{p}")
                dq.dma_start(
                    out=dnp, in_=dsc1[idx, :].rearrange("(p j) -> p j", p=128)
                )
                rcp = rec_pool.tile([128, 8], F32, tag="rcp", name=f"rcp_{qc}_{p}")
                sc = scale_pool.tile([64, 1024], F32, tag="scale",
                                     name=f"sc_{qc}_{p}")
                stack = stack_pool.tile([128, 512], MMD, tag="stack",
                                        name=f"stk_{qc}_{p}")
                stkB = tmp_pool.tile([64, 512], MMD, tag="stkB", bufs=3,
                                     name=f"skB_{qc}_{p}")

                def den_finish(idx=idx, dq=dq, dnp=dnp, rcp=rcp, sc=sc,
                               stack=stack, stkB=stkB, tmpA=tmpA, tmpB=tmpB):
                    nc.vector.reciprocal(rcp, dnp)
                    dq.dma_start(
                        out=dsc2[idx, :].rearrange("(p j) -> p j", p=128),
                        in_=rcp,
                    )
                    dq.dma_start(
                        out=sc[0:64, :],
                        in_=dsc2[idx:idx + 1, :].to_broadcast([64, 1024]),
                    )
                    nc.vector.tensor_mul(stack[0:64, :], tmpA[0:64, :],
                                         sc[0:64, 0:512])
                    nc.vector.tensor_mul(stkB[0:64, :], tmpB[0:64, :],
                                         sc[0:64, 512:1024])
                    dq.dma_start(out=stack[64:128, :], in_=stkB[0:64, :])

                pending_den[0] = den_finish
            stacks.append(stack)

            if qc == NQC - 1 and p == 2:
                fillers.append(gen_proj(qc, stacks, pair_sel=[0, 1]))
                fillers.append(proj_gens.pop(2))
                proj3c_gen = gen_proj(qc, stacks, pair_sel=[2],
                                      target=out2_ap, row_base=0)
            pull(2)

        if qc in kv_gens:
            kv_force(qc, 1000)
        drain_required()
        if qc + 1 < NQC:
            # qts for the next chunk exist now (gen_xq drained above)
            pre_emitted[(qc + 1, 0)] = emit_qk_exp(qc + 1, 0)
        stacks_store[qc] = stacks
        if qc < NQC - 1:
            proj_gens[qc] = gen_proj(qc, stacks)
        carry = [g for g in fillers if g not in must_drain]

    # drain anything still pending (proj3a / proj3c leftovers)
    for g in carry:
        for _ in g:
            pass

    # ---- tail: pair 3 of the final chunk into out3, as K=64 matmuls of
    # the UNNORMALIZED per-head y with 1/den folded into the evacuation
    # (per-partition scale from rcp8): halfA = psA*rcpA on the ACT engine,
    # pout = psB*rcpB + halfA fused on the DVE. ----
    tmpA8, tmpB8, rcp8 = last_pair
    tail_queues = [nc.sync, nc.scalar]
    slot_iter = [(ps_pool, "ps"), (st_pool, "st"), (y_pool, "y")]
    units = [(tq, ch) for tq in range(4) for ch in range(2)]
    for i, (tq, ch) in enumerate(units):
        poolA, tagA = slot_iter[(2 * i) % 3]
        poolB, tagB = slot_iter[(2 * i + 1) % 3]
        psA = poolA.tile([128, 512], F32, tag=tagA, name=f"t3a_{tq}_{ch}")
        nc.tensor.matmul(
            psA, tmpA8[0:64, tq * 128:(tq + 1) * 128],
            wp_sb[0:64, 3, ch * 512:(ch + 1) * 512],
            start=True, stop=True, skip_group_check=True,
        )
        psB = poolB.tile([128, 512], F32, tag=tagB, name=f"t3b_{tq}_{ch}")
        nc.tensor.matmul(
            psB, tmpB8[0:64, tq * 128:(tq + 1) * 128],
            wp3b[0:64, ch * 512:(ch + 1) * 512],
            start=True, stop=True, skip_group_check=True,
        )
        halfA = tmp_pool.tile([128, 512], F32, tag="tmp", name=f"ha_{tq}_{ch}")
        nc.scalar.activation(halfA, psA, Copy, scale=rcp8[:, tq:tq + 1])
        pout = pout_pool.tile([128, 512], MMD, tag="pout",
                              name=f"po3_{tq}_{ch}")
        nc.vector.scalar_tensor_tensor(
            out=pout, in0=psB, scalar=rcp8[:, 4 + tq:5 + tq], in1=halfA,
            op0=mybir.AluOpType.mult, op1=mybir.AluOpType.add,
        )
        tail_queues[i % 2].dma_start(
            out=out3_ap[tq * 128:(tq + 1) * 128, ch * 512:(ch + 1) * 512],
            in_=pout,
        )


def make_nc():
    nc = bacc.Bacc("TRN2", target_bir_lowering=False, debug=False,
                   num_devices=NCORES)
    xT = nc.dram_tensor("xT", [C, T], MMD, kind="ExternalInput")
    wqkv = nc.dram_tensor("wqkv", [C, 3 * FPC], MMD, kind="ExternalInput")
    wproj = nc.dram_tensor("wproj", [FPC, C], MMD, kind="ExternalInput")
    ea = nc.dram_tensor("ea", [T], F32, kind="ExternalInput")
    out = nc.dram_tensor("out", [T, C], MMD, kind="ExternalOutput")
    out2 = nc.dram_tensor("out2", [512, C], MMD, kind="ExternalOutput")
    out3 = nc.dram_tensor("out3", [512, C], MMD, kind="ExternalOutput")
    dsc1 = nc.dram_tensor("dsc1", [16, 1024], F32, kind="Internal")
    dsc2 = nc.dram_tensor("dsc2", [16, 1024], F32, kind="Internal")
    tri_np = np.triu(np.ones((128, 128), dtype=NPMMD))
    tri_dram = nc.inline_tensor(tri_np, name="tri_const")
    with ExitStack() as ctx:
        tc = ctx.enter_context(tile.TileContext(nc))
        tc.ctx = ctx
        build(tc, out[:, :], out2[:, :], out3[:, :], xT[:, :],
              wqkv[:, :], wproj[:, :], ea[:], tri_dram, dsc1[:, :],
              dsc2[:, :])
    nc.compile()
    return nc


def shard_inputs(x, prev_probs, W_attn, W_proj):
    in_maps = []
    for core in range(NCORES):
        b, g = divmod(core, 2)
        xT = np.ascontiguousarray(x[b].T)
        wq = W_attn[:, g * FPC:(g + 1) * FPC]
        wk = W_attn[:, C + g * FPC:C + (g + 1) * FPC]
        wv = W_attn[:, 2 * C + g * FPC:2 * C + (g + 1) * FPC]
        wqkv = np.ascontiguousarray(np.concatenate([wq, wk, wv], axis=1))
        wproj = np.ascontiguousarray(W_proj[g * FPC:(g + 1) * FPC, :])
        ea = np.power(prev_probs[b] + np.float32(1e-10), np.float32(-EPS_BIAS))
        in_maps.append(
            {
                "xT": xT.astype(NPMMD),
                "wqkv": wqkv.astype(NPMMD),
                "wproj": wproj.astype(NPMMD),
                "ea": ea.astype(np.float32),
            }
        )
    return in_maps


_CACHED_NC = None


def kernel(x, prev_probs, W_attn, W_proj, trace=False, tmpdir=None):
    global _CACHED_NC
    from concourse.bass_utils import run_bass_kernel_spmd

    x = np.asarray(x, dtype=np.float32)
    prev_probs = np.asarray(prev_probs, dtype=np.float32)
    W_attn = np.asarray(W_attn, dtype=np.float32)
    W_proj = np.asarray(W_proj, dtype=np.float32)

    if _CACHED_NC is None:
        _CACHED_NC = make_nc()
    nc = _CACHED_NC

    in_maps = shard_inputs(x, prev_probs, W_attn, W_proj)
    res = run_bass_kernel_spmd(
        nc, in_maps, core_ids=list(range(NCORES)), trace=trace, tmpdir=tmpdir
    )
    out = np.empty((B, T, C), dtype=np.float32)
    for b in range(B):
        acc = (res.results[2 * b]["out"].astype(np.float32)
               + res.results[2 * b + 1]["out"].astype(np.float32))
        for part in ("out2", "out3"):
            acc[3 * 512:] += (res.results[2 * b][part].astype(np.float32)
                              + res.results[2 * b + 1][part].astype(np.float32))
        out[b] = acc
    kernel.last_results = res
    return out
